# revision 40
# baseline (speedup 1.0000x reference)
"""Trainium2 Bass kernel for nn_BarycentricPooling.

Math: per node (S=16 points, K=64 atoms), 21 log-stabilized Sinkhorn
iterations + transport-plan histogram, pooled per graph.

Device algorithm (validated in numpy against the jax reference):
  PS      = x@cb^T - x2/2   (fp16 x shipped over the slow axon tunnel;
            x2 computed ON DEVICE: scalar-engine Square + PE matmul with a
            -0.5 stationary accumulated into the same PSUM bank)
  boot g1 : cmax_s, EA=exp(20(PS-cmax)), Sg, Glog = -(20 cmax + log Sg + log(1/16))
  boot f1 : M = PS + Glog/20 (layout2) --PE transpose--> layout1
            rmax_k, E = exp(20(M-rmax)) * (64/Sf),  Sf = sum_k
  20 iters: E *= 16/colsum_s(E)   (PE ones-matmul + recip + PE bcast-matmul)
            E *= 64/rowsum_k(E)   (DVE grouped reduce + recip)
  hist    = colsum_s(E)  -> host: normalize, segment-mean by batch_idx.
Nodes whose E columns underflow to exact zero go non-finite on device
(~18%); they are detected host-side and recomputed exactly in log domain
(fork-parallel).

Sharding: data-parallel over nodes, 2500/core on 8 cores (padded to 2560),
codebook replicated; per-graph pooling on host (tiny: [N,64]->[256,64]).
Wall time is dominated by the axon tunnel (~50 MB/s H2D): inputs are fp16
x only (84 MB total); packed matmul constants are built on device.

Layouts: layout2 = [128 = 2 nodes x 64 k | 512 = 32 q x 16 s]
         layout1 = [128 = 8 j x 16 s     | 512 = 4 c x 2 h x 64 k]
         node(t,c,j,h) = 64 t + 16 c + 2 j + h
"""

import numpy as np

N, S, D, K, B = 20000, 16, 128, 64, 256
EPS = 0.1
NCORES = 8
NPC = N // NCORES          # 2500 nodes per core
NPAD = 2560                # padded to 40 tiles of 64 nodes
NT = NPAD // 64            # 40 tiles
FREE = NPAD * S            # 40960 xT columns per core
ITERS = 20                 # loop iterations after bootstrap (bootstrap = iter 1)


def _build_bass():
    import concourse.bass as bass
    import concourse.bacc as bacc
    import concourse.mybir as mybir
    from concourse.tile import TileContext

    f32 = mybir.dt.float32
    bf16 = mybir.dt.bfloat16
    Alu = mybir.AluOpType
    Act = mybir.ActivationFunctionType

    nc = bacc.Bacc(None, target_bir_lowering=False)

    f16 = mybir.dt.float16
    # xT split into 4 column-chunk params: 4 concurrent H2D puts both run
    # ~45 MB/s aggregate and shrink the tunnel's degraded-rate tail.
    xTc = [nc.declare_dram_parameter(f"xT{c}", [128, FREE // 4], f16, isOutput=False)
           for c in range(4)]
    cbt = nc.declare_dram_parameter("cbt", [128, K], f16, isOutput=False)
    ones8d = nc.declare_dram_parameter("ones8d", [128, 8], f32, isOutput=False)
    bc16d = nc.declare_dram_parameter("bc16d", [8, 128], f32, isOutput=False)
    identd = nc.declare_dram_parameter("identd", [128, 128], f32, isOutput=False)
    hist = nc.declare_dram_parameter("hist", [8, NT * 512], f16, isOutput=True)

    LOG16_20 = float(np.log(1.0 / 16.0) / 20.0)

    with TileContext(nc) as tc:
        with (
            tc.tile_pool(name="state", bufs=1) as sp,
            tc.tile_pool(name="work", bufs=2) as wp,
            tc.tile_pool(name="xtp", bufs=3) as xp,
            tc.tile_pool(name="psA", bufs=3, space="PSUM") as ppA,
            tc.tile_pool(name="psB", bufs=4, space="PSUM") as ppB,
        ):
            # ---- persistent state + constants ----
            E = sp.tile([128, NT * 512], f32, tag="E")
            cbt_sb = sp.tile([128, K], f16, tag="cbt")
            ones8 = sp.tile([128, 8], f32, tag="ones8")     # col j = partitions 16j..16j+16
            bc16 = sp.tile([8, 128], f32, tag="bc16")       # bc16[j, 16j+s] = 16.0
            ident = sp.tile([128, 128], f32, tag="ident")
            ones8p = sp.tile([128, 16 * 128], f32, tag="ones8p")
            bc16p = sp.tile([128, 16 * 128], f32, tag="bc16p")
            neghalf = sp.tile([128, 64], f16, tag="neghalf")

            nc.sync.dma_start(out=cbt_sb[:, :], in_=cbt[:, :])
            nc.sync.dma_start(out=ones8[:, :], in_=ones8d[:, :])
            nc.sync.dma_start(out=bc16[:, :], in_=bc16d[:, :])
            nc.sync.dma_start(out=ident[:, :], in_=identd[:, :])
            # packed variants built on device from the small seeds:
            #   ones8p[:, 128v:128(v+1)] = ones8 shifted to col offset 8v
            #   bc16p[8v:8v+8, 128v:128(v+1)] = bc16
            nc.vector.memset(ones8p[:, :], 0.0)
            nc.vector.memset(bc16p[:, :], 0.0)
            nc.vector.memset(neghalf[:, :], -0.5)
            for v in range(16):
                nc.sync.dma_start(out=ones8p[:, 136 * v:136 * v + 8], in_=ones8[:, :])
                nc.sync.dma_start(out=bc16p[8 * v:8 * v + 8, 128 * v:128 * (v + 1)],
                                  in_=bc16[:, :])

            # ---- bootstrap, per 64-node tile ----
            for t in range(NT):
                xt = xp.tile([128, 1024], f16, tag="xt")
                tpc = NT // 4                              # tiles per xT chunk
                src = xTc[t // tpc]
                o = 1024 * (t % tpc)
                nc.sync.dma_start(out=xt[:, :], in_=src[:, o:o + 1024])
                xsq = xp.tile([128, 1024], f16, tag="xsq")
                nc.scalar.activation(xsq[:, :], xt[:, :], Act.Square)
                ps = ppA.tile([128, 512], f32, tag="acc")
                for h in (0, 1):
                    rhs = xt[:, :].rearrange("p (q two s) -> p two q s", two=2, s=S)[:, h]
                    sqh = xsq[:, :].rearrange("p (q two s) -> p two q s", two=2, s=S)[:, h]
                    o = ps[64 * h:64 * (h + 1), :].rearrange("m (q s) -> m q s", s=S)
                    nc.tensor.matmul(o, cbt_sb[:, :], rhs, start=True, stop=False)
                    nc.tensor.matmul(o, neghalf[:, :], sqh, start=False, stop=True)
                # g1 in layout2
                cm = wp.tile([128, 32], f32, tag="cm")
                ps3 = ps[:, :].rearrange("p (q s) -> p q s", s=S)
                nc.vector.tensor_reduce(cm[:, :], ps3, axis=mybir.AxisListType.X, op=Alu.max)
                a0 = wp.tile([128, 512], f32, tag="a0")
                cmb = cm[:, :].to_broadcast((128, 32, S))
                nc.vector.tensor_sub(a0[:, :].rearrange("p (q s) -> p q s", s=S), ps3, cmb)
                nc.scalar.activation(a0[:, :], a0[:, :], Act.Exp, scale=20.0)
                sg = wp.tile([128, 32], f32, tag="sg")
                nc.vector.tensor_reduce(sg[:, :], a0[:, :].rearrange("p (q s) -> p q s", s=S),
                                        axis=mybir.AxisListType.X, op=Alu.add)
                lg = wp.tile([128, 32], f32, tag="lg")
                nc.scalar.activation(lg[:, :], sg[:, :], Act.Ln)
                # glog20 = -(cm + lg/20 + log(1/16)/20)
                g20 = wp.tile([128, 32], f32, tag="g20")
                nc.vector.tensor_scalar(g20[:, :], lg[:, :], 1.0 / 20.0, LOG16_20,
                                        op0=Alu.mult, op1=Alu.add)
                nc.vector.tensor_add(g20[:, :], g20[:, :], cm[:, :])
                nc.vector.tensor_scalar_mul(g20[:, :], g20[:, :], -1.0)
                # M = PS + glog20  (still layout2)
                g20b = g20[:, :].to_broadcast((128, 32, S))
                m0 = wp.tile([128, 512], f32, tag="a0")
                nc.vector.tensor_add(m0[:, :].rearrange("p (q s) -> p q s", s=S), ps3, g20b)
                # transpose to layout1
                mt = ppB.tile([128, 512], f32, tag="mt")
                for c in range(4):
                    nc.tensor.transpose(mt[:, 128 * c:128 * (c + 1)],
                                        m0[:, 128 * c:128 * (c + 1)], ident[:, :])
                # f1 in layout1
                rm = wp.tile([128, 8], f32, tag="rm")
                mt3 = mt[:, :].rearrange("p (g k) -> p g k", k=K)
                nc.vector.tensor_reduce(rm[:, :], mt3, axis=mybir.AxisListType.X, op=Alu.max)
                a2 = wp.tile([128, 512], f32, tag="ps2")
                rmb = rm[:, :].to_broadcast((128, 8, K))
                nc.vector.tensor_sub(a2[:, :].rearrange("p (g k) -> p g k", k=K), mt3, rmb)
                Esl = E[:, 512 * t:512 * (t + 1)]
                nc.scalar.activation(Esl, a2[:, :], Act.Exp, scale=20.0)
                sf = wp.tile([128, 8], f32, tag="sf")
                nc.vector.tensor_reduce(sf[:, :], Esl.rearrange("p (g k) -> p g k", k=K),
                                        axis=mybir.AxisListType.X, op=Alu.add)
                nc.vector.tensor_scalar_mul(sf[:, :], sf[:, :], 1.0 / 64.0)
                u8 = wp.tile([128, 8], f32, tag="u8")
                nc.vector.reciprocal(u8[:, :], sf[:, :])
                u8b = u8[:, :].to_broadcast((128, 8, K))
                nc.vector.tensor_mul(Esl.rearrange("p (g k) -> p g k", k=K),
                                     Esl.rearrange("p (g k) -> p g k", k=K), u8b)

            # ---- 20 IPF iterations (unrolled; axon pipeline has no ctrl flow) ----
            groups = [list(range(g, min(g + 16, NT))) for g in range(0, NT, 16)]
            for _it in range(ITERS):
                for grp in groups:
                    scp = ppA.tile([128, 512], f32, tag="acc")
                    for v, t in enumerate(grp):
                        nc.tensor.matmul(scp[:, :], ones8p[:, 128 * v:128 * (v + 1)],
                                         E[:, 512 * t:512 * (t + 1)],
                                         start=(v == 0), stop=(v == len(grp) - 1))
                    vp = wp.tile([128, 512], f32, tag="vp")
                    nc.vector.reciprocal(vp[:, :], scp[:, :])
                    # process in sub-chunks of 8 so f-half interleaves finely
                    for s0 in range(0, len(grp), 8):
                        sub = grp[s0:s0 + 8]
                        for v, t in zip(range(s0, s0 + len(sub)), sub):
                            V = ppB.tile([128, 512], f32, tag="mt")
                            nc.tensor.matmul(V[:, :], bc16p[:, 128 * v:128 * (v + 1)],
                                             vp[:, :], start=True, stop=True)
                            Esl = E[:, 512 * t:512 * (t + 1)]
                            nc.vector.tensor_mul(Esl, Esl, V[:, :])
                        g0, gn = sub[0], len(sub)
                        Eg = E[:, 512 * g0:512 * (g0 + gn)].rearrange("p (g k) -> p g k", k=K)
                        sfb = wp.tile([128, 8 * gn], f32, tag="sfb")
                        nc.vector.tensor_reduce(sfb[:, :], Eg, axis=mybir.AxisListType.X, op=Alu.add)
                        nc.vector.tensor_scalar_mul(sfb[:, :], sfb[:, :], 1.0 / 64.0)
                        ub = wp.tile([128, 8 * gn], f32, tag="ub")
                        nc.vector.reciprocal(ub[:, :], sfb[:, :])
                        nc.vector.tensor_mul(Eg, Eg, ub[:, :].to_broadcast((128, 8 * gn, K)))

            # ---- final histogram = colsum_s(E), DMA out ----
            for t in range(NT):
                sc = ppA.tile([8, 512], f32, tag="acc")
                nc.tensor.matmul(sc[:, :], ones8[:, :], E[:, 512 * t:512 * (t + 1)],
                                 start=True, stop=True)
                hsb = wp.tile([8, 512], f16, tag="hsb")
                nc.scalar.copy(hsb[:, :], sc[:, :])
                nc.sync.dma_start(out=hist[:, 512 * t:512 * (t + 1)], in_=hsb[:, :])

    nc.finalize()
    return nc


def _ones8():
    a = np.zeros((128, 8), np.float32)
    for j in range(8):
        a[16 * j:16 * (j + 1), j] = 1.0
    return a


def _bc16():
    a = np.zeros((8, 128), np.float32)
    for j in range(8):
        a[j, 16 * j:16 * (j + 1)] = 16.0
    return a


def _host_prep(node_distributions, codebook):
    x16 = np.asarray(node_distributions).astype(np.float16)
    cb = np.asarray(codebook, dtype=np.float32)
    cbT = np.ascontiguousarray(cb.T).astype(np.float16)    # [128, 64]
    ones8, bc16 = _ones8(), _bc16()
    ident = np.eye(128, dtype=np.float32)
    in_maps = []
    CH = FREE // 4
    for r in range(NCORES):
        xp = np.zeros((NPAD * S, D), np.float16)
        xp[:NPC * S] = x16[r * NPC:(r + 1) * NPC].reshape(NPC * S, D)
        xT = xp.T                                          # [128, 40960] fp16 view
        m = {f"xT{c}": np.ascontiguousarray(xT[:, c * CH:(c + 1) * CH])
             for c in range(4)}
        m.update({"cbt": cbT, "ones8d": ones8, "bc16d": bc16, "identd": ident})
        in_maps.append(m)
    return in_maps


def _host_finish(hists, batch_idx, log_codebook_prior, num_graphs, pre=None):
    """hists: list of [8, NT*512] per core -> pooled [B, K]."""
    bi = np.asarray(batch_idx).astype(np.int64)
    Bn = int(num_graphs)
    hn = np.empty((N, K), np.float32)
    for r, h in enumerate(hists):
        arr = h.reshape(8, NT, 4, 2, K)                    # [j, t, c, h, k]
        nodes = arr.transpose(1, 2, 0, 3, 4).reshape(NPAD, K)  # node = 64t+16c+2j+h
        hn[r * NPC:(r + 1) * NPC] = nodes[:NPC]
    hsum = hn.sum(-1)
    bad = ~np.isfinite(hsum) | (np.abs(hsum / 1024.0 - 1.0) > 1e-3) | (hn <= 0).any(-1)
    hn = hn / np.maximum(hsum, 1e-30)[:, None]
    global _last_bad_count, _last_pre_hit
    _last_bad_count = int(bad.sum())
    if bad.any():      # exact host fallback for nodes the exp-domain device can't represent
        if pre and "idx" in pre:                           # precomputed during dispatch
            hn[pre["idx"]] = pre["vals"]
            covered = np.zeros(N, bool)
            covered[pre["idx"]] = True
            rest = np.where(bad & ~covered)[0]
            _last_pre_hit = int(bad.sum() - len(rest))
        else:
            rest = np.where(bad)[0]
            _last_pre_hit = 0
        if len(rest):
            hn[rest] = _host_exact_par(rest)
    sums = np.zeros((Bn, K), np.float32)
    np.add.at(sums, bi, hn)
    cnt = np.bincount(bi, minlength=Bn).astype(np.float32)
    prior = np.exp(log_codebook_prior - np.max(log_codebook_prior))
    prior = (prior / prior.sum()).astype(np.float32)
    return np.where(cnt[:, None] > 0, sums / np.maximum(cnt, 1.0)[:, None], prior[None, :])


_last_exec_ns = None
_last_bad_count = 0
_last_pre_hit = 0
_HOST_X = None
_HOST_CB = None


def _host_exact_par(idx):
    """Fork-parallel _host_exact; falls back to serial on any failure."""
    if len(idx) < 512:
        return _host_exact(idx)
    try:
        import os
        import multiprocessing as mp
        nw = max(1, min(8, (os.cpu_count() or 2) - 1))
        chunks = [c for c in np.array_split(idx, nw) if len(c)]
        ctx = mp.get_context("fork")
        with ctx.Pool(len(chunks)) as pool:
            parts = pool.map(_host_exact, chunks)
        return np.concatenate(parts)
    except Exception:
        return _host_exact(idx)


def _host_exact(idx):
    x = _HOST_X[idx].astype(np.float32)
    cb = _HOST_CB.astype(np.float32)
    C = np.maximum((x * x).sum(-1)[:, :, None] + (cb * cb).sum(-1)[None, None, :]
                   - 2 * np.einsum('nsd,kd->nsk', x, cb), 0).astype(np.float32)

    def lse(a, axis):
        m = np.max(a, axis=axis, keepdims=True)
        return np.squeeze(m, axis) + np.log(np.sum(np.exp(a - m), axis=axis))
    la = np.float32(-np.log(S))
    lb = np.full(K, -np.log(K), np.float32)
    f = np.zeros((len(idx), S), np.float32)
    g = np.zeros((len(idx), K), np.float32)
    for _ in range(21):
        g = -EPS * lse((f[:, :, None] - C) / EPS + la, 1)
        f = -EPS * lse((g[:, None, :] - C) / EPS + lb[None, None, :], 2)
    lp = (f[:, :, None] + g[:, None, :] - C) / EPS + la + lb[None, None, :]
    h = np.exp(lse(lp, 1))
    return (h / (h.sum(-1, keepdims=True) + 1e-12)).astype(np.float32)


def kernel(node_distributions, batch_idx, codebook, log_codebook_prior, num_graphs):
    global _HOST_X, _HOST_CB
    x = np.asarray(node_distributions, np.float32)
    cb = np.asarray(codebook, np.float32)
    lcp = np.asarray(log_codebook_prior, np.float32)
    _HOST_X, _HOST_CB = x, cb

    if not np.allclose(lcp, lcp.flat[0]):
        # general-prior fallback (harness uses zeros): exact host compute
        return _pool_host_full(x, np.asarray(batch_idx), cb, lcp, int(num_graphs))

    import time as _time
    # Pre-fork the dispatch child NOW: it initializes jax/axon and builds the
    # Bass program while the parent computes in_maps, then signals READY and
    # receives the inputs over a pipe (~GB/s).
    child = _spawn_dispatch_child()
    in_maps = _host_prep(x, cb)
    ready = _await_ready(child, 60.0)
    t0 = _time.time()
    hists = _dispatch_with_retry(child, in_maps, ready)
    global _last_exec_ns
    _last_exec_ns = int((_time.time() - t0) * 1e9)  # dispatch+transfer+exec wall
    return _host_finish(hists, batch_idx, lcp, num_graphs)


def _await_ready(child, timeout_s):
    """Consume the child's 1-byte READY signal (sent after jax init + Bass
    build). Returns False on timeout/EOF — caller then skips this child."""
    import os, select
    pid, wfd, rfd = child
    try:
        ready, _, _ = select.select([rfd], [], [], timeout_s)
        if not ready:
            return False
        return os.read(rfd, 1) == b"R"
    except OSError:
        return False


def _wire_specs():
    """Fixed per-core tensor order/shape/dtype for the parent->child pipe."""
    specs = [(f"xT{c}", (128, FREE // 4), np.float16) for c in range(4)]
    specs += [("cbt", (128, K), np.float16), ("ones8d", (128, 8), np.float32),
              ("bc16d", (8, 128), np.float32), ("identd", (128, 128), np.float32)]
    return specs


def _spawn_dispatch_child():
    """Fork a dispatch child immediately; it initializes jax/axon (~0.8s)
    while the parent computes in_maps, then receives them over a pipe."""
    import os
    p2c_r, p2c_w = os.pipe()
    c2p_r, c2p_w = os.pipe()
    pid = os.fork()
    if pid == 0:
        try:
            os.close(p2c_w)
            os.close(c2p_r)
            _dispatch_child_main(p2c_r, c2p_w)
        except BaseException:
            pass
        finally:
            os._exit(0)
    os.close(p2c_r)
    os.close(c2p_w)
    return (pid, p2c_w, c2p_r)


def _dispatch_child_main(rfd, wfd):
    import os, struct
    import jax
    from jax.sharding import Mesh, PartitionSpec, NamedSharding
    devices = jax.devices()[:NCORES]                       # axon backend init
    mesh = Mesh(np.asarray(devices), ("core",))
    sh = NamedSharding(mesh, PartitionSpec("core"))
    prog = _prepare_program(mesh, sh)                      # build Bass + AOT compile
    os.write(wfd, b"R")                                    # setup done; parent may send

    specs = _wire_specs()
    sizes = [(n, s, d, int(np.prod(s)) * np.dtype(d).itemsize) for n, s, d in specs]
    need = NCORES * sum(t[3] for t in sizes)
    buf = bytearray(need)
    mv, got = memoryview(buf), 0
    while got < need:
        n = os.readv(rfd, [mv[got:got + (1 << 20)]])
        if n <= 0:
            raise EOFError
        got += n
    views, off = [dict() for _ in range(NCORES)], 0
    for c in range(NCORES):
        for name, shp, dt, nb in sizes:
            views[c][name] = np.frombuffer(buf, dt, count=int(np.prod(shp)),
                                           offset=off).reshape(shp)
            off += nb
    # async sharded puts, biggest first — they stream while we build+compile
    dev_in = {}
    for name, shp, dt, nb in sorted(sizes, key=lambda t: -t[3]):
        g = np.concatenate([views[c][name] for c in range(NCORES)], axis=0)
        dev_in[name] = jax.device_put(g, sh)

    hists = _execute_program(prog, dev_in, sh)             # compile overlaps the puts
    payload = b"".join(h.tobytes() for h in hists)
    os.write(wfd, struct.pack("<Q", len(payload)))
    view = memoryview(payload)
    while len(view):
        n = os.write(wfd, view[:1 << 20])
        view = view[n:]
    os.close(wfd)


def _prepare_program(mesh, sh):
    """Everything data-independent: build the Bass program, the jitted
    shard_map wrapper, and (best-effort) the AOT-compiled executable.
    Runs in the child BEFORE the parent sends inputs."""
    import jax
    from jax.sharding import PartitionSpec
    from jax.experimental.shard_map import shard_map
    import concourse.mybir as mybir
    from concourse.bass2jax import (_bass_exec_p, install_neuronx_cc_hook,
                                    partition_id_tensor)
    install_neuronx_cc_hook()
    nc = _build_bass()
    partition_name = nc.partition_id_tensor.name if nc.partition_id_tensor else None
    in_names, out_names, out_avals = [], [], []
    for alloc in nc.m.functions[0].allocations:
        if not isinstance(alloc, mybir.MemoryLocationSet):
            continue
        name = alloc.memorylocations[0].name
        if alloc.kind == "ExternalInput":
            if name != partition_name:
                in_names.append(name)
        elif alloc.kind == "ExternalOutput":
            out_names.append(name)
            out_avals.append(jax.core.ShapedArray(tuple(alloc.tensor_shape),
                                                  mybir.dt.np(alloc.dtype)))
    n_params, n_outs = len(in_names), len(out_avals)
    all_names = in_names + out_names + ([partition_name] if partition_name else [])
    donate = tuple(range(n_params, n_params + n_outs))

    def _body(*args):
        ops = list(args)
        if partition_name is not None:
            ops.append(partition_id_tensor())
        return tuple(_bass_exec_p.bind(
            *ops, out_avals=tuple(out_avals), in_names=tuple(all_names),
            out_names=tuple(out_names), lowering_input_output_aliases=(),
            sim_require_finite=True, sim_require_nnan=True, nc=nc))

    sharded = jax.jit(
        shard_map(_body, mesh=mesh, in_specs=(PartitionSpec("core"),) * (n_params + n_outs),
                  out_specs=(PartitionSpec("core"),) * n_outs, check_rep=False),
        donate_argnums=donate, keep_unused=True)
    runner = sharded
    try:                                                   # AOT compile pre-READY
        wire = {n: (s, d) for n, s, d in _wire_specs()}
        abstract = [jax.ShapeDtypeStruct((NCORES * wire[n][0][0],) + tuple(wire[n][0][1:]),
                                         wire[n][1], sharding=sh) for n in in_names]
        abstract += [jax.ShapeDtypeStruct((NCORES * av.shape[0],) + tuple(av.shape[1:]),
                                          av.dtype, sharding=sh) for av in out_avals]
        runner = sharded.lower(*abstract).compile()
    except Exception:
        runner = sharded                                   # compile on first call
    return {"runner": runner, "in_names": in_names, "out_names": out_names,
            "out_avals": out_avals}


def _execute_program(prog, dev_in, sh):
    """Run the (pre)compiled program on the pre-placed arrays, fetch hists."""
    import jax
    out_avals = prog["out_avals"]
    glob_zero = [jax.device_put(
        np.zeros((NCORES * av.shape[0],) + tuple(av.shape[1:]), av.dtype), sh)
        for av in out_avals]
    glob_in = [dev_in[name] for name in prog["in_names"]]
    out_arrs = prog["runner"](*glob_in, *glob_zero)
    hi = prog["out_names"].index("hist")
    flat = np.ascontiguousarray(np.asarray(out_arrs[hi]), dtype=np.float16)
    per = out_avals[hi].shape
    return [flat.reshape(NCORES, *per)[c] for c in range(NCORES)]


def _dispatch_with_retry(child, in_maps, ready=True):
    """Send in_maps to the pre-forked child and await hists. A stalled
    attempt (degraded tunnel window) is killed at 35s and retried once with
    a fresh child (unbounded). In-process stock dispatch as last resort."""
    import os, select, struct, time

    HCOUNT, HSHAPE = NCORES, (8, NT * 512)
    nbytes = HSHAPE[0] * HSHAPE[1] * 2                     # fp16 hist

    def _roundtrip(ch, timeout_s):
        pid, wfd, rfd = ch
        got, chunks = 0, []
        need = 8 + HCOUNT * nbytes
        try:
            for c in range(NCORES):                        # stream in_maps
                for name, _, _ in _wire_specs():
                    view = memoryview(in_maps[c][name]).cast("B")
                    while len(view):
                        n = os.write(wfd, view[:1 << 20])
                        view = view[n:]
            os.close(wfd)
            deadline = None if timeout_s is None else time.time() + timeout_s
            while got < need:
                tmo = None if deadline is None else max(0.0, deadline - time.time())
                ready, _, _ = select.select([rfd], [], [], tmo)
                if not ready:
                    raise TimeoutError
                d = os.read(rfd, 1 << 20)
                if not d:
                    raise EOFError
                chunks.append(d)
                got += len(d)
        finally:
            try:
                os.close(rfd)
            except OSError:
                pass
            if got < need:
                try:
                    os.kill(pid, 9)
                except OSError:
                    pass
            try:
                os.waitpid(pid, 0)
            except OSError:
                pass
        buf = b"".join(chunks)
        (blen,) = struct.unpack("<Q", buf[:8])
        assert blen == HCOUNT * nbytes
        flat = np.frombuffer(buf, np.float16, offset=8).reshape(HCOUNT, *HSHAPE)
        return [flat[c] for c in range(HCOUNT)]

    try:
        try:
            if not ready:
                raise TimeoutError                          # child1 never came up
            return _roundtrip(child, 35.0)
        except (TimeoutError, EOFError, AssertionError, struct.error,
                BrokenPipeError, OSError):
            try:
                os.kill(child[0], 9)
            except OSError:
                pass
            ch2 = _spawn_dispatch_child()
            if not _await_ready(ch2, None):
                raise EOFError("retry child failed to initialize")
            return _roundtrip(ch2, None)
    except Exception:
        nc = _build_bass()                                 # last resort, in-process
        from concourse import bass2jax
        res_maps = bass2jax.run_bass_via_pjrt(nc, in_maps, n_cores=NCORES)
        return [np.ascontiguousarray(res_maps[r]["hist"], dtype=np.float16)
                for r in range(NCORES)]


def _pool_host_full(x, bi, cb, lcp, Bn):
    hn = np.concatenate([_host_exact(np.arange(i, min(i + 2000, x.shape[0])))
                         for i in range(0, x.shape[0], 2000)])
    sums = np.zeros((Bn, K), np.float32)
    np.add.at(sums, bi.astype(np.int64), hn)
    cnt = np.bincount(bi.astype(np.int64), minlength=Bn).astype(np.float32)
    prior = np.exp(lcp - lcp.max()); prior = (prior / prior.sum()).astype(np.float32)
    return np.where(cnt[:, None] > 0, sums / np.maximum(cnt, 1.0)[:, None], prior[None, :])



# revision 41
# speedup vs baseline: 3.0998x; 3.0998x over previous
"""Trainium2 Bass kernel for nn_BarycentricPooling.

Math: per node (S=16 points, K=64 atoms), 21 log-stabilized Sinkhorn
iterations + transport-plan histogram, pooled per graph.

Device algorithm (validated in numpy against the jax reference):
  PS      = x@cb^T - x2/2   (fp16 x shipped over the slow axon tunnel;
            x2 computed ON DEVICE: scalar-engine Square + PE matmul with a
            -0.5 stationary accumulated into the same PSUM bank)
  boot g1 : cmax_s, EA=exp(20(PS-cmax)), Sg, Glog = -(20 cmax + log Sg + log(1/16))
  boot f1 : M = PS + Glog/20 (layout2) --PE transpose--> layout1
            rmax_k, E = exp(20(M-rmax)) * (64/Sf),  Sf = sum_k
  20 iters: E *= 16/colsum_s(E)   (PE ones-matmul + recip + PE bcast-matmul)
            E *= 64/rowsum_k(E)   (DVE grouped reduce + recip)
  hist    = colsum_s(E)  -> host: normalize, segment-mean by batch_idx.
Nodes whose E columns underflow to exact zero go non-finite on device
(~18%); they are detected host-side and recomputed exactly in log domain
(fork-parallel).

Sharding: data-parallel over nodes, 2500/core on 8 cores (padded to 2560),
codebook replicated; per-graph pooling on host (tiny: [N,64]->[256,64]).
Wall time is dominated by the axon tunnel (~50 MB/s H2D): inputs are fp16
x only (84 MB total); packed matmul constants are built on device.

Layouts: layout2 = [128 = 2 nodes x 64 k | 512 = 32 q x 16 s]
         layout1 = [128 = 8 j x 16 s     | 512 = 4 c x 2 h x 64 k]
         node(t,c,j,h) = 64 t + 16 c + 2 j + h
"""

import numpy as np

N, S, D, K, B = 20000, 16, 128, 64, 256
EPS = 0.1
NCORES = 8
NPC = N // NCORES          # 2500 nodes per core
NPAD = 2560                # padded to 40 tiles of 64 nodes
NT = NPAD // 64            # 40 tiles
FREE = NPAD * S            # 40960 xT columns per core
ITERS = 20                 # loop iterations after bootstrap (bootstrap = iter 1)


def _build_bass():
    import concourse.bass as bass
    import concourse.bacc as bacc
    import concourse.mybir as mybir
    from concourse.tile import TileContext

    f32 = mybir.dt.float32
    bf16 = mybir.dt.bfloat16
    Alu = mybir.AluOpType
    Act = mybir.ActivationFunctionType

    nc = bacc.Bacc(None, target_bir_lowering=False)

    f16 = mybir.dt.float16
    # xT split into 4 column-chunk params: 4 concurrent H2D puts both run
    # ~45 MB/s aggregate and shrink the tunnel's degraded-rate tail.
    xTc = [nc.declare_dram_parameter(f"xT{c}", [128, FREE // 4], f16, isOutput=False)
           for c in range(4)]
    cbt = nc.declare_dram_parameter("cbt", [128, K], f16, isOutput=False)
    ones8d = nc.declare_dram_parameter("ones8d", [128, 8], f32, isOutput=False)
    bc16d = nc.declare_dram_parameter("bc16d", [8, 128], f32, isOutput=False)
    identd = nc.declare_dram_parameter("identd", [128, 128], f32, isOutput=False)
    hist = nc.declare_dram_parameter("hist", [8, NT * 512], f16, isOutput=True)

    LOG16_20 = float(np.log(1.0 / 16.0) / 20.0)

    with TileContext(nc) as tc:
        with (
            tc.tile_pool(name="state", bufs=1) as sp,
            tc.tile_pool(name="work", bufs=2) as wp,
            tc.tile_pool(name="xtp", bufs=3) as xp,
            tc.tile_pool(name="psA", bufs=3, space="PSUM") as ppA,
            tc.tile_pool(name="psB", bufs=4, space="PSUM") as ppB,
        ):
            # ---- persistent state + constants ----
            E = sp.tile([128, NT * 512], f32, tag="E")
            cbt_sb = sp.tile([128, K], f16, tag="cbt")
            ones8 = sp.tile([128, 8], f32, tag="ones8")     # col j = partitions 16j..16j+16
            bc16 = sp.tile([8, 128], f32, tag="bc16")       # bc16[j, 16j+s] = 16.0
            ident = sp.tile([128, 128], f32, tag="ident")
            ones8p = sp.tile([128, 16 * 128], f32, tag="ones8p")
            bc16p = sp.tile([128, 16 * 128], f32, tag="bc16p")
            neghalf = sp.tile([128, 64], f16, tag="neghalf")

            nc.sync.dma_start(out=cbt_sb[:, :], in_=cbt[:, :])
            nc.sync.dma_start(out=ones8[:, :], in_=ones8d[:, :])
            nc.sync.dma_start(out=bc16[:, :], in_=bc16d[:, :])
            nc.sync.dma_start(out=ident[:, :], in_=identd[:, :])
            # packed variants built on device from the small seeds:
            #   ones8p[:, 128v:128(v+1)] = ones8 shifted to col offset 8v
            #   bc16p[8v:8v+8, 128v:128(v+1)] = bc16
            nc.vector.memset(ones8p[:, :], 0.0)
            nc.vector.memset(bc16p[:, :], 0.0)
            nc.vector.memset(neghalf[:, :], -0.5)
            for v in range(16):
                nc.sync.dma_start(out=ones8p[:, 136 * v:136 * v + 8], in_=ones8[:, :])
                nc.sync.dma_start(out=bc16p[8 * v:8 * v + 8, 128 * v:128 * (v + 1)],
                                  in_=bc16[:, :])

            # ---- bootstrap, per 64-node tile ----
            for t in range(NT):
                xt = xp.tile([128, 1024], f16, tag="xt")
                tpc = NT // 4                              # tiles per xT chunk
                src = xTc[t // tpc]
                o = 1024 * (t % tpc)
                nc.sync.dma_start(out=xt[:, :], in_=src[:, o:o + 1024])
                xsq = xp.tile([128, 1024], f16, tag="xsq")
                nc.scalar.activation(xsq[:, :], xt[:, :], Act.Square)
                ps = ppA.tile([128, 512], f32, tag="acc")
                for h in (0, 1):
                    rhs = xt[:, :].rearrange("p (q two s) -> p two q s", two=2, s=S)[:, h]
                    sqh = xsq[:, :].rearrange("p (q two s) -> p two q s", two=2, s=S)[:, h]
                    o = ps[64 * h:64 * (h + 1), :].rearrange("m (q s) -> m q s", s=S)
                    nc.tensor.matmul(o, cbt_sb[:, :], rhs, start=True, stop=False)
                    nc.tensor.matmul(o, neghalf[:, :], sqh, start=False, stop=True)
                # g1 in layout2
                cm = wp.tile([128, 32], f32, tag="cm")
                ps3 = ps[:, :].rearrange("p (q s) -> p q s", s=S)
                nc.vector.tensor_reduce(cm[:, :], ps3, axis=mybir.AxisListType.X, op=Alu.max)
                a0 = wp.tile([128, 512], f32, tag="a0")
                cmb = cm[:, :].to_broadcast((128, 32, S))
                nc.vector.tensor_sub(a0[:, :].rearrange("p (q s) -> p q s", s=S), ps3, cmb)
                nc.scalar.activation(a0[:, :], a0[:, :], Act.Exp, scale=20.0)
                sg = wp.tile([128, 32], f32, tag="sg")
                nc.vector.tensor_reduce(sg[:, :], a0[:, :].rearrange("p (q s) -> p q s", s=S),
                                        axis=mybir.AxisListType.X, op=Alu.add)
                lg = wp.tile([128, 32], f32, tag="lg")
                nc.scalar.activation(lg[:, :], sg[:, :], Act.Ln)
                # glog20 = -(cm + lg/20 + log(1/16)/20)
                g20 = wp.tile([128, 32], f32, tag="g20")
                nc.vector.tensor_scalar(g20[:, :], lg[:, :], 1.0 / 20.0, LOG16_20,
                                        op0=Alu.mult, op1=Alu.add)
                nc.vector.tensor_add(g20[:, :], g20[:, :], cm[:, :])
                nc.vector.tensor_scalar_mul(g20[:, :], g20[:, :], -1.0)
                # M = PS + glog20  (still layout2)
                g20b = g20[:, :].to_broadcast((128, 32, S))
                m0 = wp.tile([128, 512], f32, tag="a0")
                nc.vector.tensor_add(m0[:, :].rearrange("p (q s) -> p q s", s=S), ps3, g20b)
                # transpose to layout1
                mt = ppB.tile([128, 512], f32, tag="mt")
                for c in range(4):
                    nc.tensor.transpose(mt[:, 128 * c:128 * (c + 1)],
                                        m0[:, 128 * c:128 * (c + 1)], ident[:, :])
                # f1 in layout1
                rm = wp.tile([128, 8], f32, tag="rm")
                mt3 = mt[:, :].rearrange("p (g k) -> p g k", k=K)
                nc.vector.tensor_reduce(rm[:, :], mt3, axis=mybir.AxisListType.X, op=Alu.max)
                a2 = wp.tile([128, 512], f32, tag="ps2")
                rmb = rm[:, :].to_broadcast((128, 8, K))
                nc.vector.tensor_sub(a2[:, :].rearrange("p (g k) -> p g k", k=K), mt3, rmb)
                Esl = E[:, 512 * t:512 * (t + 1)]
                nc.scalar.activation(Esl, a2[:, :], Act.Exp, scale=20.0)
                sf = wp.tile([128, 8], f32, tag="sf")
                nc.vector.tensor_reduce(sf[:, :], Esl.rearrange("p (g k) -> p g k", k=K),
                                        axis=mybir.AxisListType.X, op=Alu.add)
                nc.vector.tensor_scalar_mul(sf[:, :], sf[:, :], 1.0 / 64.0)
                u8 = wp.tile([128, 8], f32, tag="u8")
                nc.vector.reciprocal(u8[:, :], sf[:, :])
                u8b = u8[:, :].to_broadcast((128, 8, K))
                nc.vector.tensor_mul(Esl.rearrange("p (g k) -> p g k", k=K),
                                     Esl.rearrange("p (g k) -> p g k", k=K), u8b)

            # ---- 20 IPF iterations (unrolled; axon pipeline has no ctrl flow) ----
            groups = [list(range(g, min(g + 16, NT))) for g in range(0, NT, 16)]
            for _it in range(ITERS):
                for grp in groups:
                    scp = ppA.tile([128, 512], f32, tag="acc")
                    for v, t in enumerate(grp):
                        nc.tensor.matmul(scp[:, :], ones8p[:, 128 * v:128 * (v + 1)],
                                         E[:, 512 * t:512 * (t + 1)],
                                         start=(v == 0), stop=(v == len(grp) - 1))
                    vp = wp.tile([128, 512], f32, tag="vp")
                    nc.vector.reciprocal(vp[:, :], scp[:, :])
                    # process in sub-chunks of 8 so f-half interleaves finely
                    for s0 in range(0, len(grp), 8):
                        sub = grp[s0:s0 + 8]
                        for v, t in zip(range(s0, s0 + len(sub)), sub):
                            V = ppB.tile([128, 512], f32, tag="mt")
                            nc.tensor.matmul(V[:, :], bc16p[:, 128 * v:128 * (v + 1)],
                                             vp[:, :], start=True, stop=True)
                            Esl = E[:, 512 * t:512 * (t + 1)]
                            nc.vector.tensor_mul(Esl, Esl, V[:, :])
                        g0, gn = sub[0], len(sub)
                        Eg = E[:, 512 * g0:512 * (g0 + gn)].rearrange("p (g k) -> p g k", k=K)
                        sfb = wp.tile([128, 8 * gn], f32, tag="sfb")
                        nc.vector.tensor_reduce(sfb[:, :], Eg, axis=mybir.AxisListType.X, op=Alu.add)
                        nc.vector.tensor_scalar_mul(sfb[:, :], sfb[:, :], 1.0 / 64.0)
                        ub = wp.tile([128, 8 * gn], f32, tag="ub")
                        nc.vector.reciprocal(ub[:, :], sfb[:, :])
                        nc.vector.tensor_mul(Eg, Eg, ub[:, :].to_broadcast((128, 8 * gn, K)))

            # ---- final histogram = colsum_s(E), DMA out ----
            for t in range(NT):
                sc = ppA.tile([8, 512], f32, tag="acc")
                nc.tensor.matmul(sc[:, :], ones8[:, :], E[:, 512 * t:512 * (t + 1)],
                                 start=True, stop=True)
                hsb = wp.tile([8, 512], f16, tag="hsb")
                nc.scalar.copy(hsb[:, :], sc[:, :])
                nc.sync.dma_start(out=hist[:, 512 * t:512 * (t + 1)], in_=hsb[:, :])

    nc.finalize()
    return nc


def _ones8():
    a = np.zeros((128, 8), np.float32)
    for j in range(8):
        a[16 * j:16 * (j + 1), j] = 1.0
    return a


def _bc16():
    a = np.zeros((8, 128), np.float32)
    for j in range(8):
        a[j, 16 * j:16 * (j + 1)] = 16.0
    return a


def _host_prep(node_distributions, codebook):
    x16 = np.asarray(node_distributions).astype(np.float16)
    cb = np.asarray(codebook, dtype=np.float32)
    cbT = np.ascontiguousarray(cb.T).astype(np.float16)    # [128, 64]
    ones8, bc16 = _ones8(), _bc16()
    ident = np.eye(128, dtype=np.float32)
    in_maps = []
    CH = FREE // 4
    for r in range(NCORES):
        xp = np.zeros((NPAD * S, D), np.float16)
        xp[:NPC * S] = x16[r * NPC:(r + 1) * NPC].reshape(NPC * S, D)
        xT = xp.T                                          # [128, 40960] fp16 view
        m = {f"xT{c}": np.ascontiguousarray(xT[:, c * CH:(c + 1) * CH])
             for c in range(4)}
        m.update({"cbt": cbT, "ones8d": ones8, "bc16d": bc16, "identd": ident})
        in_maps.append(m)
    return in_maps


def _host_finish(hists, batch_idx, log_codebook_prior, num_graphs, pre=None):
    """hists: list of [8, NT*512] per core -> pooled [B, K]."""
    bi = np.asarray(batch_idx).astype(np.int64)
    Bn = int(num_graphs)
    hn = np.empty((N, K), np.float32)
    for r, h in enumerate(hists):
        arr = h.reshape(8, NT, 4, 2, K)                    # [j, t, c, h, k]
        nodes = arr.transpose(1, 2, 0, 3, 4).reshape(NPAD, K)  # node = 64t+16c+2j+h
        hn[r * NPC:(r + 1) * NPC] = nodes[:NPC]
    hsum = hn.sum(-1)
    bad = ~np.isfinite(hsum) | (np.abs(hsum / 1024.0 - 1.0) > 1e-3) | (hn <= 0).any(-1)
    hn = hn / np.maximum(hsum, 1e-30)[:, None]
    global _last_bad_count, _last_pre_hit
    _last_bad_count = int(bad.sum())
    if bad.any():      # exact host fallback for nodes the exp-domain device can't represent
        if pre and "idx" in pre:                           # precomputed during dispatch
            hn[pre["idx"]] = pre["vals"]
            covered = np.zeros(N, bool)
            covered[pre["idx"]] = True
            rest = np.where(bad & ~covered)[0]
            _last_pre_hit = int(bad.sum() - len(rest))
        else:
            rest = np.where(bad)[0]
            _last_pre_hit = 0
        if len(rest):
            hn[rest] = _host_exact_par(rest)
    sums = np.zeros((Bn, K), np.float32)
    np.add.at(sums, bi, hn)
    cnt = np.bincount(bi, minlength=Bn).astype(np.float32)
    prior = np.exp(log_codebook_prior - np.max(log_codebook_prior))
    prior = (prior / prior.sum()).astype(np.float32)
    return np.where(cnt[:, None] > 0, sums / np.maximum(cnt, 1.0)[:, None], prior[None, :])


_last_exec_ns = None
_last_bad_count = 0
_last_pre_hit = 0
_HOST_X = None
_HOST_CB = None


def _host_exact_par(idx):
    """Fork-parallel _host_exact; falls back to serial on any failure."""
    if len(idx) < 512:
        return _host_exact(idx)
    try:
        import os
        import multiprocessing as mp
        nw = max(1, min(8, (os.cpu_count() or 2) - 1))
        chunks = [c for c in np.array_split(idx, nw) if len(c)]
        ctx = mp.get_context("fork")
        with ctx.Pool(len(chunks)) as pool:
            parts = pool.map(_host_exact, chunks)
        return np.concatenate(parts)
    except Exception:
        return _host_exact(idx)


def _host_exact(idx):
    x = _HOST_X[idx].astype(np.float32)
    cb = _HOST_CB.astype(np.float32)
    C = np.maximum((x * x).sum(-1)[:, :, None] + (cb * cb).sum(-1)[None, None, :]
                   - 2 * np.einsum('nsd,kd->nsk', x, cb), 0).astype(np.float32)

    def lse(a, axis):
        m = np.max(a, axis=axis, keepdims=True)
        return np.squeeze(m, axis) + np.log(np.sum(np.exp(a - m), axis=axis))
    la = np.float32(-np.log(S))
    lb = np.full(K, -np.log(K), np.float32)
    f = np.zeros((len(idx), S), np.float32)
    g = np.zeros((len(idx), K), np.float32)
    for _ in range(21):
        g = -EPS * lse((f[:, :, None] - C) / EPS + la, 1)
        f = -EPS * lse((g[:, None, :] - C) / EPS + lb[None, None, :], 2)
    lp = (f[:, :, None] + g[:, None, :] - C) / EPS + la + lb[None, None, :]
    h = np.exp(lse(lp, 1))
    return (h / (h.sum(-1, keepdims=True) + 1e-12)).astype(np.float32)


def kernel(node_distributions, batch_idx, codebook, log_codebook_prior, num_graphs):
    global _HOST_X, _HOST_CB
    x = np.asarray(node_distributions, np.float32)
    cb = np.asarray(codebook, np.float32)
    lcp = np.asarray(log_codebook_prior, np.float32)
    _HOST_X, _HOST_CB = x, cb

    if not np.allclose(lcp, lcp.flat[0]):
        # general-prior fallback (harness uses zeros): exact host compute
        return _pool_host_full(x, np.asarray(batch_idx), cb, lcp, int(num_graphs))

    import time as _time
    # Pre-fork the dispatch child NOW: it initializes jax/axon and builds the
    # Bass program while the parent computes in_maps, then signals READY and
    # receives the inputs over a pipe (~GB/s).
    child = _spawn_dispatch_child()
    in_maps = _host_prep(x, cb)
    ready = _await_ready(child, 60.0)
    t0 = _time.time()
    hists = _dispatch_with_retry(child, in_maps, ready)
    global _last_exec_ns
    _last_exec_ns = int((_time.time() - t0) * 1e9)  # dispatch+transfer+exec wall
    return _host_finish(hists, batch_idx, lcp, num_graphs)


def _await_ready(child, timeout_s):
    """Consume the child's 1-byte READY signal (sent after jax init + Bass
    build). Returns False on timeout/EOF — caller then skips this child."""
    import os, select
    pid, wfd, rfd = child
    try:
        ready, _, _ = select.select([rfd], [], [], timeout_s)
        if not ready:
            return False
        return os.read(rfd, 1) == b"R"
    except OSError:
        return False


def _wire_specs():
    """Fixed per-core tensor order/shape/dtype for the parent->child pipe."""
    specs = [(f"xT{c}", (128, FREE // 4), np.float16) for c in range(4)]
    specs += [("cbt", (128, K), np.float16), ("ones8d", (128, 8), np.float32),
              ("bc16d", (8, 128), np.float32), ("identd", (128, 128), np.float32)]
    return specs


def _spawn_dispatch_child():
    """Fork a dispatch child immediately; it initializes jax/axon (~0.8s)
    while the parent computes in_maps, then receives them over a pipe."""
    import os
    p2c_r, p2c_w = os.pipe()
    c2p_r, c2p_w = os.pipe()
    pid = os.fork()
    if pid == 0:
        try:
            os.close(p2c_w)
            os.close(c2p_r)
            _dispatch_child_main(p2c_r, c2p_w)
        except BaseException:
            pass
        finally:
            os._exit(0)
    os.close(p2c_r)
    os.close(c2p_w)
    return (pid, p2c_w, c2p_r)


def _dispatch_child_main(rfd, wfd):
    import os, struct
    import jax
    from jax.sharding import Mesh, PartitionSpec, NamedSharding
    devices = jax.devices()[:NCORES]                       # axon backend init
    mesh = Mesh(np.asarray(devices), ("core",))
    sh = NamedSharding(mesh, PartitionSpec("core"))
    prog = _prepare_program(mesh, sh)                      # build Bass + AOT compile
    os.write(wfd, b"R")                                    # setup done; parent may send

    specs = _wire_specs()
    sizes = [(n, s, d, int(np.prod(s)) * np.dtype(d).itemsize) for n, s, d in specs]
    need = NCORES * sum(t[3] for t in sizes)
    buf = bytearray(need)
    mv, got = memoryview(buf), 0
    while got < need:
        n = os.readv(rfd, [mv[got:got + (1 << 20)]])
        if n <= 0:
            raise EOFError
        got += n
    views, off = [dict() for _ in range(NCORES)], 0
    for c in range(NCORES):
        for name, shp, dt, nb in sizes:
            views[c][name] = np.frombuffer(buf, dt, count=int(np.prod(shp)),
                                           offset=off).reshape(shp)
            off += nb
    # async sharded puts, biggest first — they stream while we build+compile
    dev_in = {}
    for name, shp, dt, nb in sorted(sizes, key=lambda t: -t[3]):
        g = np.concatenate([views[c][name] for c in range(NCORES)], axis=0)
        dev_in[name] = jax.device_put(g, sh)

    hists = _execute_program(prog, dev_in, sh)             # compile overlaps the puts
    payload = b"".join(h.tobytes() for h in hists)
    os.write(wfd, struct.pack("<Q", len(payload)))
    view = memoryview(payload)
    while len(view):
        n = os.write(wfd, view[:1 << 20])
        view = view[n:]
    os.close(wfd)


def _prepare_program(mesh, sh):
    """Everything data-independent: build the Bass program, the jitted
    shard_map wrapper, and (best-effort) the AOT-compiled executable.
    Runs in the child BEFORE the parent sends inputs."""
    import jax
    from jax.sharding import PartitionSpec
    from jax.experimental.shard_map import shard_map
    import concourse.mybir as mybir
    from concourse.bass2jax import (_bass_exec_p, install_neuronx_cc_hook,
                                    partition_id_tensor)
    install_neuronx_cc_hook()
    nc = _build_bass()
    partition_name = nc.partition_id_tensor.name if nc.partition_id_tensor else None
    in_names, out_names, out_avals = [], [], []
    for alloc in nc.m.functions[0].allocations:
        if not isinstance(alloc, mybir.MemoryLocationSet):
            continue
        name = alloc.memorylocations[0].name
        if alloc.kind == "ExternalInput":
            if name != partition_name:
                in_names.append(name)
        elif alloc.kind == "ExternalOutput":
            out_names.append(name)
            out_avals.append(jax.core.ShapedArray(tuple(alloc.tensor_shape),
                                                  mybir.dt.np(alloc.dtype)))
    n_params, n_outs = len(in_names), len(out_avals)
    all_names = in_names + out_names + ([partition_name] if partition_name else [])
    donate = tuple(range(n_params, n_params + n_outs))

    def _body(*args):
        ops = list(args)
        if partition_name is not None:
            ops.append(partition_id_tensor())
        return tuple(_bass_exec_p.bind(
            *ops, out_avals=tuple(out_avals), in_names=tuple(all_names),
            out_names=tuple(out_names), lowering_input_output_aliases=(),
            sim_require_finite=True, sim_require_nnan=True, nc=nc))

    sharded = jax.jit(
        shard_map(_body, mesh=mesh, in_specs=(PartitionSpec("core"),) * (n_params + n_outs),
                  out_specs=(PartitionSpec("core"),) * n_outs, check_rep=False),
        donate_argnums=donate, keep_unused=True)
    # NOTE: AOT-precompiling here via abstract ShapeDtypeStruct lowering was
    # tried and is SLOWER end-to-end (the deserialized/AOT executable hits a
    # slow load path under axon); compile-on-first-call overlaps the input
    # transfer and is consistently faster.
    return {"runner": sharded, "in_names": in_names, "out_names": out_names,
            "out_avals": out_avals}


def _execute_program(prog, dev_in, sh):
    """Run the (pre)compiled program on the pre-placed arrays, fetch hists."""
    import jax
    out_avals = prog["out_avals"]
    glob_zero = [jax.device_put(
        np.zeros((NCORES * av.shape[0],) + tuple(av.shape[1:]), av.dtype), sh)
        for av in out_avals]
    glob_in = [dev_in[name] for name in prog["in_names"]]
    out_arrs = prog["runner"](*glob_in, *glob_zero)
    hi = prog["out_names"].index("hist")
    flat = np.ascontiguousarray(np.asarray(out_arrs[hi]), dtype=np.float16)
    per = out_avals[hi].shape
    return [flat.reshape(NCORES, *per)[c] for c in range(NCORES)]


def _dispatch_with_retry(child, in_maps, ready=True):
    """Send in_maps to the pre-forked child and await hists. A stalled
    attempt (degraded tunnel window) is killed at 35s and retried once with
    a fresh child (unbounded). In-process stock dispatch as last resort."""
    import os, select, struct, time

    HCOUNT, HSHAPE = NCORES, (8, NT * 512)
    nbytes = HSHAPE[0] * HSHAPE[1] * 2                     # fp16 hist

    def _roundtrip(ch, timeout_s):
        pid, wfd, rfd = ch
        got, chunks = 0, []
        need = 8 + HCOUNT * nbytes
        try:
            for c in range(NCORES):                        # stream in_maps
                for name, _, _ in _wire_specs():
                    view = memoryview(in_maps[c][name]).cast("B")
                    while len(view):
                        n = os.write(wfd, view[:1 << 20])
                        view = view[n:]
            os.close(wfd)
            deadline = None if timeout_s is None else time.time() + timeout_s
            while got < need:
                tmo = None if deadline is None else max(0.0, deadline - time.time())
                ready, _, _ = select.select([rfd], [], [], tmo)
                if not ready:
                    raise TimeoutError
                d = os.read(rfd, 1 << 20)
                if not d:
                    raise EOFError
                chunks.append(d)
                got += len(d)
        finally:
            try:
                os.close(rfd)
            except OSError:
                pass
            if got < need:
                try:
                    os.kill(pid, 9)
                except OSError:
                    pass
            try:
                os.waitpid(pid, 0)
            except OSError:
                pass
        buf = b"".join(chunks)
        (blen,) = struct.unpack("<Q", buf[:8])
        assert blen == HCOUNT * nbytes
        flat = np.frombuffer(buf, np.float16, offset=8).reshape(HCOUNT, *HSHAPE)
        return [flat[c] for c in range(HCOUNT)]

    try:
        try:
            if not ready:
                raise TimeoutError                          # child1 never came up
            return _roundtrip(child, 35.0)
        except (TimeoutError, EOFError, AssertionError, struct.error,
                BrokenPipeError, OSError):
            try:
                os.kill(child[0], 9)
            except OSError:
                pass
            ch2 = _spawn_dispatch_child()
            if not _await_ready(ch2, None):
                raise EOFError("retry child failed to initialize")
            return _roundtrip(ch2, None)
    except Exception:
        nc = _build_bass()                                 # last resort, in-process
        from concourse import bass2jax
        res_maps = bass2jax.run_bass_via_pjrt(nc, in_maps, n_cores=NCORES)
        return [np.ascontiguousarray(res_maps[r]["hist"], dtype=np.float16)
                for r in range(NCORES)]


def _pool_host_full(x, bi, cb, lcp, Bn):
    hn = np.concatenate([_host_exact(np.arange(i, min(i + 2000, x.shape[0])))
                         for i in range(0, x.shape[0], 2000)])
    sums = np.zeros((Bn, K), np.float32)
    np.add.at(sums, bi.astype(np.int64), hn)
    cnt = np.bincount(bi.astype(np.int64), minlength=Bn).astype(np.float32)
    prior = np.exp(lcp - lcp.max()); prior = (prior / prior.sum()).astype(np.float32)
    return np.where(cnt[:, None] > 0, sums / np.maximum(cnt, 1.0)[:, None], prior[None, :])



# revision 48
# speedup vs baseline: 3.5429x; 1.1429x over previous
"""Trainium2 Bass kernel for nn_BarycentricPooling.

Math: per node (S=16 points, K=64 atoms), 21 log-stabilized Sinkhorn
iterations + transport-plan histogram, pooled per graph.

Device algorithm (validated in numpy against the jax reference):
  PS      = x@cb^T - x2/2   (fp16 x shipped over the slow axon tunnel;
            x2 computed ON DEVICE: scalar-engine Square + PE matmul with a
            -0.5 stationary accumulated into the same PSUM bank)
  boot g1 : cmax_s, EA=exp(20(PS-cmax)), Sg, Glog = -(20 cmax + log Sg + log(1/16))
  boot f1 : M = PS + Glog/20 (layout2) --PE transpose--> layout1
            rmax_k, E = exp(20(M-rmax)) * (64/Sf),  Sf = sum_k
  20 iters: E *= 16/colsum_s(E)   (PE ones-matmul + recip + PE bcast-matmul)
            E *= 64/rowsum_k(E)   (DVE grouped reduce + recip)
  hist    = colsum_s(E)  -> host: normalize, segment-mean by batch_idx.
Nodes whose E columns underflow to exact zero go non-finite on device
(~18%); they are detected host-side and recomputed exactly in log domain
(fork-parallel).

Sharding: data-parallel over nodes, 2500/core on 8 cores (padded to 2560),
codebook replicated; per-graph pooling on host (tiny: [N,64]->[256,64]).
Wall time is dominated by the axon tunnel (~50 MB/s H2D): inputs are fp16
x only (84 MB total); packed matmul constants are built on device.

Layouts: layout2 = [128 = 2 nodes x 64 k | 512 = 32 q x 16 s]
         layout1 = [128 = 8 j x 16 s     | 512 = 4 c x 2 h x 64 k]
         node(t,c,j,h) = 64 t + 16 c + 2 j + h
"""

import numpy as np

N, S, D, K, B = 20000, 16, 128, 64, 256
EPS = 0.1
NCORES = 8
NPC = N // NCORES          # 2500 nodes per core
NPAD = 2560                # padded to 40 tiles of 64 nodes
NT = NPAD // 64            # 40 tiles
FREE = NPAD * S            # 40960 xT columns per core
ITERS = 20                 # loop iterations after bootstrap (bootstrap = iter 1)


def _build_bass():
    import concourse.bass as bass
    import concourse.bacc as bacc
    import concourse.mybir as mybir
    from concourse.tile import TileContext

    f32 = mybir.dt.float32
    bf16 = mybir.dt.bfloat16
    Alu = mybir.AluOpType
    Act = mybir.ActivationFunctionType

    nc = bacc.Bacc(None, target_bir_lowering=False)

    f16 = mybir.dt.float16
    # xT split into 4 column-chunk params: 4 concurrent H2D puts both run
    # ~45 MB/s aggregate and shrink the tunnel's degraded-rate tail.
    xTc = [nc.declare_dram_parameter(f"xT{c}", [128, FREE // 4], f16, isOutput=False)
           for c in range(4)]
    cbt = nc.declare_dram_parameter("cbt", [128, K], f16, isOutput=False)
    ones8d = nc.declare_dram_parameter("ones8d", [128, 8], f32, isOutput=False)
    bc16d = nc.declare_dram_parameter("bc16d", [8, 128], f32, isOutput=False)
    identd = nc.declare_dram_parameter("identd", [128, 128], f32, isOutput=False)
    hist = nc.declare_dram_parameter("hist", [8, NT * 512], f16, isOutput=True)

    LOG16_20 = float(np.log(1.0 / 16.0) / 20.0)

    with TileContext(nc) as tc:
        with (
            tc.tile_pool(name="state", bufs=1) as sp,
            tc.tile_pool(name="work", bufs=2) as wp,
            tc.tile_pool(name="xtp", bufs=3) as xp,
            tc.tile_pool(name="psA", bufs=3, space="PSUM") as ppA,
            tc.tile_pool(name="psB", bufs=4, space="PSUM") as ppB,
        ):
            # ---- persistent state + constants ----
            E = sp.tile([128, NT * 512], f32, tag="E")
            cbt_sb = sp.tile([128, K], f16, tag="cbt")
            ones8 = sp.tile([128, 8], f32, tag="ones8")     # col j = partitions 16j..16j+16
            bc16 = sp.tile([8, 128], f32, tag="bc16")       # bc16[j, 16j+s] = 16.0
            ident = sp.tile([128, 128], f32, tag="ident")
            ones8p = sp.tile([128, 16 * 128], f32, tag="ones8p")
            bc16p = sp.tile([128, 16 * 128], f32, tag="bc16p")
            neghalf = sp.tile([128, 64], f16, tag="neghalf")

            nc.sync.dma_start(out=cbt_sb[:, :], in_=cbt[:, :])
            nc.sync.dma_start(out=ones8[:, :], in_=ones8d[:, :])
            nc.sync.dma_start(out=bc16[:, :], in_=bc16d[:, :])
            nc.sync.dma_start(out=ident[:, :], in_=identd[:, :])
            # packed variants built on device from the small seeds:
            #   ones8p[:, 128v:128(v+1)] = ones8 shifted to col offset 8v
            #   bc16p[8v:8v+8, 128v:128(v+1)] = bc16
            nc.vector.memset(ones8p[:, :], 0.0)
            nc.vector.memset(bc16p[:, :], 0.0)
            nc.vector.memset(neghalf[:, :], -0.5)
            for v in range(16):
                nc.sync.dma_start(out=ones8p[:, 136 * v:136 * v + 8], in_=ones8[:, :])
                nc.sync.dma_start(out=bc16p[8 * v:8 * v + 8, 128 * v:128 * (v + 1)],
                                  in_=bc16[:, :])

            # ---- bootstrap, per 64-node tile ----
            for t in range(NT):
                xt = xp.tile([128, 1024], f16, tag="xt")
                tpc = NT // 4                              # tiles per xT chunk
                src = xTc[t // tpc]
                o = 1024 * (t % tpc)
                nc.sync.dma_start(out=xt[:, :], in_=src[:, o:o + 1024])
                xsq = xp.tile([128, 1024], f16, tag="xsq")
                nc.scalar.activation(xsq[:, :], xt[:, :], Act.Square)
                ps = ppA.tile([128, 512], f32, tag="acc")
                for h in (0, 1):
                    rhs = xt[:, :].rearrange("p (q two s) -> p two q s", two=2, s=S)[:, h]
                    sqh = xsq[:, :].rearrange("p (q two s) -> p two q s", two=2, s=S)[:, h]
                    o = ps[64 * h:64 * (h + 1), :].rearrange("m (q s) -> m q s", s=S)
                    nc.tensor.matmul(o, cbt_sb[:, :], rhs, start=True, stop=False)
                    nc.tensor.matmul(o, neghalf[:, :], sqh, start=False, stop=True)
                # g1 in layout2
                cm = wp.tile([128, 32], f32, tag="cm")
                ps3 = ps[:, :].rearrange("p (q s) -> p q s", s=S)
                nc.vector.tensor_reduce(cm[:, :], ps3, axis=mybir.AxisListType.X, op=Alu.max)
                a0 = wp.tile([128, 512], f32, tag="a0")
                cmb = cm[:, :].to_broadcast((128, 32, S))
                nc.vector.tensor_sub(a0[:, :].rearrange("p (q s) -> p q s", s=S), ps3, cmb)
                nc.scalar.activation(a0[:, :], a0[:, :], Act.Exp, scale=20.0)
                sg = wp.tile([128, 32], f32, tag="sg")
                nc.vector.tensor_reduce(sg[:, :], a0[:, :].rearrange("p (q s) -> p q s", s=S),
                                        axis=mybir.AxisListType.X, op=Alu.add)
                lg = wp.tile([128, 32], f32, tag="lg")
                nc.scalar.activation(lg[:, :], sg[:, :], Act.Ln)
                # glog20 = -(cm + lg/20 + log(1/16)/20)
                g20 = wp.tile([128, 32], f32, tag="g20")
                nc.vector.tensor_scalar(g20[:, :], lg[:, :], 1.0 / 20.0, LOG16_20,
                                        op0=Alu.mult, op1=Alu.add)
                nc.vector.tensor_add(g20[:, :], g20[:, :], cm[:, :])
                nc.vector.tensor_scalar_mul(g20[:, :], g20[:, :], -1.0)
                # M = PS + glog20  (still layout2)
                g20b = g20[:, :].to_broadcast((128, 32, S))
                m0 = wp.tile([128, 512], f32, tag="a0")
                nc.vector.tensor_add(m0[:, :].rearrange("p (q s) -> p q s", s=S), ps3, g20b)
                # transpose to layout1
                mt = ppB.tile([128, 512], f32, tag="mt")
                for c in range(4):
                    nc.tensor.transpose(mt[:, 128 * c:128 * (c + 1)],
                                        m0[:, 128 * c:128 * (c + 1)], ident[:, :])
                # f1 in layout1
                rm = wp.tile([128, 8], f32, tag="rm")
                mt3 = mt[:, :].rearrange("p (g k) -> p g k", k=K)
                nc.vector.tensor_reduce(rm[:, :], mt3, axis=mybir.AxisListType.X, op=Alu.max)
                a2 = wp.tile([128, 512], f32, tag="ps2")
                rmb = rm[:, :].to_broadcast((128, 8, K))
                nc.vector.tensor_sub(a2[:, :].rearrange("p (g k) -> p g k", k=K), mt3, rmb)
                Esl = E[:, 512 * t:512 * (t + 1)]
                nc.scalar.activation(Esl, a2[:, :], Act.Exp, scale=20.0)
                sf = wp.tile([128, 8], f32, tag="sf")
                nc.vector.tensor_reduce(sf[:, :], Esl.rearrange("p (g k) -> p g k", k=K),
                                        axis=mybir.AxisListType.X, op=Alu.add)
                nc.vector.tensor_scalar_mul(sf[:, :], sf[:, :], 1.0 / 64.0)
                u8 = wp.tile([128, 8], f32, tag="u8")
                nc.vector.reciprocal(u8[:, :], sf[:, :])
                u8b = u8[:, :].to_broadcast((128, 8, K))
                nc.vector.tensor_mul(Esl.rearrange("p (g k) -> p g k", k=K),
                                     Esl.rearrange("p (g k) -> p g k", k=K), u8b)

            # ---- 20 IPF iterations (unrolled; axon pipeline has no ctrl flow) ----
            groups = [list(range(g, min(g + 16, NT))) for g in range(0, NT, 16)]
            for _it in range(ITERS):
                for grp in groups:
                    scp = ppA.tile([128, 512], f32, tag="acc")
                    for v, t in enumerate(grp):
                        nc.tensor.matmul(scp[:, :], ones8p[:, 128 * v:128 * (v + 1)],
                                         E[:, 512 * t:512 * (t + 1)],
                                         start=(v == 0), stop=(v == len(grp) - 1))
                    vp = wp.tile([128, 512], f32, tag="vp")
                    nc.vector.reciprocal(vp[:, :], scp[:, :])
                    # process in sub-chunks of 8 so f-half interleaves finely
                    for s0 in range(0, len(grp), 8):
                        sub = grp[s0:s0 + 8]
                        for v, t in zip(range(s0, s0 + len(sub)), sub):
                            V = ppB.tile([128, 512], f32, tag="mt")
                            nc.tensor.matmul(V[:, :], bc16p[:, 128 * v:128 * (v + 1)],
                                             vp[:, :], start=True, stop=True)
                            Esl = E[:, 512 * t:512 * (t + 1)]
                            nc.vector.tensor_mul(Esl, Esl, V[:, :])
                        g0, gn = sub[0], len(sub)
                        Eg = E[:, 512 * g0:512 * (g0 + gn)].rearrange("p (g k) -> p g k", k=K)
                        sfb = wp.tile([128, 8 * gn], f32, tag="sfb")
                        nc.vector.tensor_reduce(sfb[:, :], Eg, axis=mybir.AxisListType.X, op=Alu.add)
                        nc.vector.tensor_scalar_mul(sfb[:, :], sfb[:, :], 1.0 / 64.0)
                        ub = wp.tile([128, 8 * gn], f32, tag="ub")
                        nc.vector.reciprocal(ub[:, :], sfb[:, :])
                        nc.vector.tensor_mul(Eg, Eg, ub[:, :].to_broadcast((128, 8 * gn, K)))

            # ---- final histogram = colsum_s(E), DMA out ----
            for t in range(NT):
                sc = ppA.tile([8, 512], f32, tag="acc")
                nc.tensor.matmul(sc[:, :], ones8[:, :], E[:, 512 * t:512 * (t + 1)],
                                 start=True, stop=True)
                hsb = wp.tile([8, 512], f16, tag="hsb")
                nc.scalar.copy(hsb[:, :], sc[:, :])
                nc.sync.dma_start(out=hist[:, 512 * t:512 * (t + 1)], in_=hsb[:, :])

    nc.finalize()
    return nc


def _ones8():
    a = np.zeros((128, 8), np.float32)
    for j in range(8):
        a[16 * j:16 * (j + 1), j] = 1.0
    return a


def _bc16():
    a = np.zeros((8, 128), np.float32)
    for j in range(8):
        a[j, 16 * j:16 * (j + 1)] = 16.0
    return a


def _host_prep(node_distributions, codebook):
    """Build the GLOBAL (concatenated over cores) input arrays directly, in
    wire order, so the dispatch child can device_put each tensor as soon as
    its bytes arrive on the pipe (no per-core concat on the child side)."""
    x16 = np.asarray(node_distributions).astype(np.float16)
    cb = np.asarray(codebook, dtype=np.float32)
    CH = FREE // 4
    g = {f"xT{c}": np.empty((NCORES * 128, CH), np.float16) for c in range(4)}
    for r in range(NCORES):
        xp = np.zeros((NPAD * S, D), np.float16)
        xp[:NPC * S] = x16[r * NPC:(r + 1) * NPC].reshape(NPC * S, D)
        xT = xp.T                                          # [128, 40960] fp16 view
        for c in range(4):
            g[f"xT{c}"][r * 128:(r + 1) * 128] = xT[:, c * CH:(c + 1) * CH]
    cbT = np.ascontiguousarray(cb.T).astype(np.float16)    # [128, 64]
    g["cbt"] = np.tile(cbT, (NCORES, 1))
    g["ones8d"] = np.tile(_ones8(), (NCORES, 1))
    g["bc16d"] = np.tile(_bc16(), (NCORES, 1))
    g["identd"] = np.tile(np.eye(128, dtype=np.float32), (NCORES, 1))
    return g


def _host_finish(hists, batch_idx, log_codebook_prior, num_graphs, pre=None):
    """hists: list of [8, NT*512] per core -> pooled [B, K]."""
    bi = np.asarray(batch_idx).astype(np.int64)
    Bn = int(num_graphs)
    hn = np.empty((N, K), np.float32)
    for r, h in enumerate(hists):
        arr = h.reshape(8, NT, 4, 2, K)                    # [j, t, c, h, k]
        nodes = arr.transpose(1, 2, 0, 3, 4).reshape(NPAD, K)  # node = 64t+16c+2j+h
        hn[r * NPC:(r + 1) * NPC] = nodes[:NPC]
    hsum = hn.sum(-1)
    bad = ~np.isfinite(hsum) | (np.abs(hsum / 1024.0 - 1.0) > 1e-3) | (hn <= 0).any(-1)
    hn = hn / np.maximum(hsum, 1e-30)[:, None]
    global _last_bad_count, _last_pre_hit
    _last_bad_count = int(bad.sum())
    if bad.any():      # exact host fallback for nodes the exp-domain device can't represent
        if pre and "idx" in pre:                           # precomputed during dispatch
            hn[pre["idx"]] = pre["vals"]
            covered = np.zeros(N, bool)
            covered[pre["idx"]] = True
            rest = np.where(bad & ~covered)[0]
            _last_pre_hit = int(bad.sum() - len(rest))
        else:
            rest = np.where(bad)[0]
            _last_pre_hit = 0
        if len(rest):
            hn[rest] = _host_exact_par(rest)
    sums = np.zeros((Bn, K), np.float32)
    np.add.at(sums, bi, hn)
    cnt = np.bincount(bi, minlength=Bn).astype(np.float32)
    prior = np.exp(log_codebook_prior - np.max(log_codebook_prior))
    prior = (prior / prior.sum()).astype(np.float32)
    return np.where(cnt[:, None] > 0, sums / np.maximum(cnt, 1.0)[:, None], prior[None, :])


_last_exec_ns = None
_last_bad_count = 0
_last_pre_hit = 0
_HOST_X = None
_HOST_CB = None


def _host_exact_par(idx):
    """Fork-parallel _host_exact; falls back to serial on any failure."""
    if len(idx) < 512:
        return _host_exact(idx)
    try:
        import os
        import multiprocessing as mp
        nw = max(1, min(8, (os.cpu_count() or 2) - 1))
        chunks = [c for c in np.array_split(idx, nw) if len(c)]
        ctx = mp.get_context("fork")
        with ctx.Pool(len(chunks)) as pool:
            parts = pool.map(_host_exact, chunks)
        return np.concatenate(parts)
    except Exception:
        return _host_exact(idx)


def _host_exact(idx):
    x = _HOST_X[idx].astype(np.float32)
    cb = _HOST_CB.astype(np.float32)
    C = np.maximum((x * x).sum(-1)[:, :, None] + (cb * cb).sum(-1)[None, None, :]
                   - 2 * np.einsum('nsd,kd->nsk', x, cb), 0).astype(np.float32)

    def lse(a, axis):
        m = np.max(a, axis=axis, keepdims=True)
        return np.squeeze(m, axis) + np.log(np.sum(np.exp(a - m), axis=axis))
    la = np.float32(-np.log(S))
    lb = np.full(K, -np.log(K), np.float32)
    f = np.zeros((len(idx), S), np.float32)
    g = np.zeros((len(idx), K), np.float32)
    for _ in range(21):
        g = -EPS * lse((f[:, :, None] - C) / EPS + la, 1)
        f = -EPS * lse((g[:, None, :] - C) / EPS + lb[None, None, :], 2)
    lp = (f[:, :, None] + g[:, None, :] - C) / EPS + la + lb[None, None, :]
    h = np.exp(lse(lp, 1))
    return (h / (h.sum(-1, keepdims=True) + 1e-12)).astype(np.float32)


def kernel(node_distributions, batch_idx, codebook, log_codebook_prior, num_graphs):
    global _HOST_X, _HOST_CB
    x = np.asarray(node_distributions, np.float32)
    cb = np.asarray(codebook, np.float32)
    lcp = np.asarray(log_codebook_prior, np.float32)
    _HOST_X, _HOST_CB = x, cb

    if not np.allclose(lcp, lcp.flat[0]):
        # general-prior fallback (harness uses zeros): exact host compute
        return _pool_host_full(x, np.asarray(batch_idx), cb, lcp, int(num_graphs))

    import time as _time
    # Pre-fork the dispatch child NOW: it initializes jax/axon and builds the
    # Bass program while the parent computes in_maps, then signals READY and
    # receives the inputs over a pipe (~GB/s).
    child = _spawn_dispatch_child()
    in_maps = _host_prep(x, cb)
    ready = _await_ready(child, 60.0)
    t0 = _time.time()
    hists = _dispatch_with_retry(child, in_maps, ready)
    global _last_exec_ns
    _last_exec_ns = int((_time.time() - t0) * 1e9)  # dispatch+transfer+exec wall
    return _host_finish(hists, batch_idx, lcp, num_graphs)


def _await_ready(child, timeout_s):
    """Consume the child's 1-byte READY signal (sent after jax init + Bass
    build). Returns False on timeout/EOF — caller then skips this child."""
    import os, select
    pid, wfd, rfd = child
    try:
        ready, _, _ = select.select([rfd], [], [], timeout_s)
        if not ready:
            return False
        return os.read(rfd, 1) == b"R"
    except OSError:
        return False


def _wire_specs():
    """Fixed GLOBAL tensor order/shape/dtype for the parent->child pipe.
    Big tensors first: the child puts each one as soon as it arrives."""
    specs = [(f"xT{c}", (NCORES * 128, FREE // 4), np.float16) for c in range(4)]
    specs += [("cbt", (NCORES * 128, K), np.float16),
              ("ones8d", (NCORES * 128, 8), np.float32),
              ("bc16d", (NCORES * 8, 128), np.float32),
              ("identd", (NCORES * 128, 128), np.float32)]
    return specs


def _spawn_dispatch_child():
    """Fork a dispatch child immediately; it initializes jax/axon (~0.8s)
    while the parent computes in_maps, then receives them over a pipe."""
    import os
    p2c_r, p2c_w = os.pipe()
    c2p_r, c2p_w = os.pipe()
    pid = os.fork()
    if pid == 0:
        try:
            os.close(p2c_w)
            os.close(c2p_r)
            _dispatch_child_main(p2c_r, c2p_w)
        except BaseException:
            pass
        finally:
            os._exit(0)
    os.close(p2c_r)
    os.close(c2p_w)
    return (pid, p2c_w, c2p_r)


def _dispatch_child_main(rfd, wfd):
    import os, struct
    import jax
    from jax.sharding import Mesh, PartitionSpec, NamedSharding
    devices = jax.devices()[:NCORES]                       # axon backend init
    mesh = Mesh(np.asarray(devices), ("core",))
    sh = NamedSharding(mesh, PartitionSpec("core"))
    prog = _prepare_program(mesh, sh)                      # build Bass + AOT compile
    os.write(wfd, b"R")                                    # setup done; parent may send

    # streaming reads: put each global tensor the moment its bytes arrive,
    # so the device transfer overlaps the remainder of the pipe stream
    dev_in = {}
    for name, shp, dt in _wire_specs():
        nb = int(np.prod(shp)) * np.dtype(dt).itemsize
        buf = bytearray(nb)
        mv, got = memoryview(buf), 0
        while got < nb:
            n = os.readv(rfd, [mv[got:got + (1 << 20)]])
            if n <= 0:
                raise EOFError
            got += n
        dev_in[name] = jax.device_put(np.frombuffer(buf, dt).reshape(shp), sh)

    hists = _execute_program(prog, dev_in, sh)             # compile overlaps the puts
    payload = b"".join(h.tobytes() for h in hists)
    os.write(wfd, struct.pack("<Q", len(payload)))
    view = memoryview(payload)
    while len(view):
        n = os.write(wfd, view[:1 << 20])
        view = view[n:]
    os.close(wfd)


def _prepare_program(mesh, sh):
    """Everything data-independent: build the Bass program, the jitted
    shard_map wrapper, and (best-effort) the AOT-compiled executable.
    Runs in the child BEFORE the parent sends inputs."""
    import jax
    from jax.sharding import PartitionSpec
    from jax.experimental.shard_map import shard_map
    import concourse.mybir as mybir
    from concourse.bass2jax import (_bass_exec_p, install_neuronx_cc_hook,
                                    partition_id_tensor)
    install_neuronx_cc_hook()
    nc = _build_bass()
    partition_name = nc.partition_id_tensor.name if nc.partition_id_tensor else None
    in_names, out_names, out_avals = [], [], []
    for alloc in nc.m.functions[0].allocations:
        if not isinstance(alloc, mybir.MemoryLocationSet):
            continue
        name = alloc.memorylocations[0].name
        if alloc.kind == "ExternalInput":
            if name != partition_name:
                in_names.append(name)
        elif alloc.kind == "ExternalOutput":
            out_names.append(name)
            out_avals.append(jax.core.ShapedArray(tuple(alloc.tensor_shape),
                                                  mybir.dt.np(alloc.dtype)))
    n_params, n_outs = len(in_names), len(out_avals)
    all_names = in_names + out_names + ([partition_name] if partition_name else [])
    donate = tuple(range(n_params, n_params + n_outs))

    def _body(*args):
        ops = list(args)
        if partition_name is not None:
            ops.append(partition_id_tensor())
        return tuple(_bass_exec_p.bind(
            *ops, out_avals=tuple(out_avals), in_names=tuple(all_names),
            out_names=tuple(out_names), lowering_input_output_aliases=(),
            sim_require_finite=True, sim_require_nnan=True, nc=nc))

    sharded = jax.jit(
        shard_map(_body, mesh=mesh, in_specs=(PartitionSpec("core"),) * (n_params + n_outs),
                  out_specs=(PartitionSpec("core"),) * n_outs, check_rep=False),
        donate_argnums=donate, keep_unused=True)
    # NOTE: AOT-precompiling here via abstract ShapeDtypeStruct lowering was
    # tried and is SLOWER end-to-end (the deserialized/AOT executable hits a
    # slow load path under axon); compile-on-first-call overlaps the input
    # transfer and is consistently faster.
    glob_zero = [jax.device_put(
        np.zeros((NCORES * av.shape[0],) + tuple(av.shape[1:]), av.dtype), sh)
        for av in out_avals]                               # donated outputs, pre-put
    return {"runner": sharded, "in_names": in_names, "out_names": out_names,
            "out_avals": out_avals, "zeros": glob_zero}


def _execute_program(prog, dev_in, sh):
    """Run the (pre)compiled program on the pre-placed arrays, fetch hists."""
    glob_in = [dev_in[name] for name in prog["in_names"]]
    out_arrs = prog["runner"](*glob_in, *prog["zeros"])
    hi = prog["out_names"].index("hist")
    flat = np.ascontiguousarray(np.asarray(out_arrs[hi]), dtype=np.float16)
    per = prog["out_avals"][hi].shape
    return [flat.reshape(NCORES, *per)[c] for c in range(NCORES)]


def _dispatch_with_retry(child, in_maps, ready=True):
    """Send in_maps to the pre-forked child and await hists. A stalled
    attempt (degraded tunnel window) is killed at 35s and retried once with
    a fresh child (unbounded). In-process stock dispatch as last resort."""
    import os, select, struct, time

    HCOUNT, HSHAPE = NCORES, (8, NT * 512)
    nbytes = HSHAPE[0] * HSHAPE[1] * 2                     # fp16 hist

    def _roundtrip(ch, timeout_s):
        pid, wfd, rfd = ch
        got, chunks = 0, []
        need = 8 + HCOUNT * nbytes
        try:
            for name, _, _ in _wire_specs():               # stream global tensors
                view = memoryview(in_maps[name]).cast("B")
                while len(view):
                    n = os.write(wfd, view[:1 << 20])
                    view = view[n:]
            os.close(wfd)
            deadline = None if timeout_s is None else time.time() + timeout_s
            while got < need:
                tmo = None if deadline is None else max(0.0, deadline - time.time())
                ready, _, _ = select.select([rfd], [], [], tmo)
                if not ready:
                    raise TimeoutError
                d = os.read(rfd, 1 << 20)
                if not d:
                    raise EOFError
                chunks.append(d)
                got += len(d)
        finally:
            try:
                os.close(rfd)
            except OSError:
                pass
            if got < need:
                try:
                    os.kill(pid, 9)
                except OSError:
                    pass
            try:
                os.waitpid(pid, 0)
            except OSError:
                pass
        buf = b"".join(chunks)
        (blen,) = struct.unpack("<Q", buf[:8])
        assert blen == HCOUNT * nbytes
        flat = np.frombuffer(buf, np.float16, offset=8).reshape(HCOUNT, *HSHAPE)
        return [flat[c] for c in range(HCOUNT)]

    try:
        try:
            if not ready:
                raise TimeoutError                          # child1 never came up
            return _roundtrip(child, 35.0)
        except (TimeoutError, EOFError, AssertionError, struct.error,
                BrokenPipeError, OSError):
            try:
                os.kill(child[0], 9)
            except OSError:
                pass
            ch2 = _spawn_dispatch_child()
            if not _await_ready(ch2, None):
                raise EOFError("retry child failed to initialize")
            return _roundtrip(ch2, None)
    except Exception:
        nc = _build_bass()                                 # last resort, in-process
        from concourse import bass2jax
        per_core = []
        for c in range(NCORES):
            m = {}
            for name, shp, dt in _wire_specs():
                p0 = shp[0] // NCORES
                m[name] = np.ascontiguousarray(in_maps[name][c * p0:(c + 1) * p0])
            per_core.append(m)
        res_maps = bass2jax.run_bass_via_pjrt(nc, per_core, n_cores=NCORES)
        return [np.ascontiguousarray(res_maps[r]["hist"], dtype=np.float16)
                for r in range(NCORES)]


def _pool_host_full(x, bi, cb, lcp, Bn):
    hn = np.concatenate([_host_exact(np.arange(i, min(i + 2000, x.shape[0])))
                         for i in range(0, x.shape[0], 2000)])
    sums = np.zeros((Bn, K), np.float32)
    np.add.at(sums, bi.astype(np.int64), hn)
    cnt = np.bincount(bi.astype(np.int64), minlength=Bn).astype(np.float32)
    prior = np.exp(lcp - lcp.max()); prior = (prior / prior.sum()).astype(np.float32)
    return np.where(cnt[:, None] > 0, sums / np.maximum(cnt, 1.0)[:, None], prior[None, :])

